# revision 1
# baseline (speedup 1.0000x reference)
"""GraphSAGE 2-layer GNN kernel for 8 TRN2 NeuronCores.

Strategy: destination-shard the 100K nodes across 8 cores (12.5K each).
Layer-1 messages x[src] are materialized host-side into a dense bf16 table in
(degree-class, segment)-packed layout, so the device only does dense streaming
DMA + DVE segment reductions + PE matmuls.  h is AllGathered (f32) across
cores; layer-2 messages are fetched from the shared h table with 128-row
indirect DMAs, then reduced/transformed identically.
"""
import sys
sys.path.insert(0, '/opt/trn_rl_repo')
import numpy as np
import ml_dtypes

import concourse.bass as bass
import concourse.tile as tile
from concourse import bacc, mybir
from concourse.bass_utils import run_bass_kernel_spmd
from concourse.masks import make_identity

N_CORES = 8
N_NODES = 100000
D = 128
SHARD = N_NODES // N_CORES  # 12500
CLASSES = [2, 4, 6, 8, 10, 12, 14, 16, 20, 24, 32, 48, 64, 96, 128]
MC1 = 160   # msg cols per chunk, layer 1 (bf16)
MC2 = 80    # msg cols per chunk, layer 2 (f32)

BF16 = ml_dtypes.bfloat16


def _class_of(deg):
    for L in CLASSES:
        if deg <= L:
            return L
    raise AssertionError(f"degree {deg} exceeds max class")


def _pack_cores(per_core_dsts_deg, mc_list):
    """Pack every core's dst segments into ONE common (class, slot, partition)
    layout so the SPMD program is identical across cores.

    per_core_dsts_deg: list (len 8) of [(node, deg)] lists.
    Returns (slots_per_class, blocks_per_core, plans) where plans is shared.
    """
    per_core_members = []
    for dd in per_core_dsts_deg:
        m = {L: [] for L in CLASSES}
        for node, deg in dd:
            m[_class_of(deg)].append(node)
        per_core_members.append(m)

    slots_per_class = {}
    for L in CLASSES:
        n = max(len(m[L]) for m in per_core_members)
        slots_per_class[L] = (n + 127) // 128

    blocks_per_core = []
    for m in per_core_members:
        blocks = []       # (L, [node or -1]*128) per (class, slot)
        for L in CLASSES:
            n_slots = slots_per_class[L]
            nodes = m[L] + [-1] * (n_slots * 128 - len(m[L]))
            for s in range(n_slots):
                blocks.append((L, nodes[s * 128:(s + 1) * 128]))
        blocks_per_core.append(blocks)

    class_of_block = []
    for L in CLASSES:
        class_of_block += [L] * slots_per_class[L]

    plans = []
    for mc in mc_list:
        plan = []          # (msg_col, L, block_index)
        cur = 0
        for b, L in enumerate(class_of_block):
            if cur % mc + L > mc:
                cur += mc - (cur % mc)    # pad to chunk boundary
            plan.append((cur, L, b))
            cur += L
        m_total = cur + ((-cur) % mc)
        plans.append((plan, m_total))
    return blocks_per_core, plans


def _build_tables(x, edge_index):
    """All host-side preprocessing. Returns per-core input dicts + metadata."""
    src = np.asarray(edge_index[0], dtype=np.int64)
    dst = np.asarray(edge_index[1], dtype=np.int64)
    deg = np.bincount(dst, minlength=N_NODES).astype(np.int64)

    # adjacency in CSR-by-dst: edges sorted by dst
    order = np.argsort(dst, kind='stable')
    src_sorted = src[order]
    dst_starts = np.zeros(N_NODES + 1, dtype=np.int64)
    np.cumsum(deg, out=dst_starts[1:])

    per_core_dd = []
    for c in range(N_CORES):
        lo, hi = c * SHARD, (c + 1) * SHARD
        per_core_dd.append([(int(n), int(deg[n])) for n in range(lo, hi)])
    blocks_per_core, plans = _pack_cores(per_core_dd, [MC1, MC2])
    cores = [{"blocks": b, "plans": plans} for b in blocks_per_core]

    nseg_max = len(blocks_per_core[0])
    (plan1, m1_max), (plan2, m2_max) = plans
    shard_rows = nseg_max * 128 + 128          # + one zero-row block

    x_bf = x.astype(BF16)
    invdeg = 1.0 / np.maximum(deg.astype(np.float32), 1.0)

    # global node -> h_full row id
    node_row = np.full(N_NODES, -1, dtype=np.int64)
    for c, ci in enumerate(cores):
        for b, (_L, nodes) in enumerate(ci["blocks"]):
            for p, n in enumerate(nodes):
                if n >= 0:
                    node_row[n] = c * shard_rows + b * 128 + p
    assert (node_row >= 0).all()

    for c, ci in enumerate(cores):
        blocks = ci["blocks"]
        (plan1, _m1), (plan2, _m2) = ci["plans"]
        zero_row = c * shard_rows + nseg_max * 128

        # layer-1 message table: slot (p, col) -> x[src] (bf16), 0 for pads
        slot_src = np.full((128, m1_max), -1, dtype=np.int64)
        # layer-2 gather indices: slot (p, col) -> h_full row
        idx2 = np.full((128, m2_max), zero_row, dtype=np.int32)

        node_of = np.full((128, nseg_max), -1, dtype=np.int64)
        for (col1, L1c, b), (col2, L2c, b2) in zip(plan1, plan2):
            assert b == b2 and L1c == L2c
            L, nodes = blocks[b]
            for p, n in enumerate(nodes):
                node_of[p, b] = n
                if n < 0:
                    continue
                s0, s1 = dst_starts[n], dst_starts[n + 1]
                srcs = src_sorted[s0:s1]
                k = len(srcs)
                slot_src[p, col1:col1 + k] = srcs
                idx2[p, col2:col2 + k] = node_row[srcs]

        t1 = np.zeros((128, m1_max, D), dtype=BF16)
        valid = slot_src >= 0
        t1[valid] = x_bf[slot_src[valid]]

        inv_tile = np.ones((128, nseg_max), dtype=np.float32)
        xdT = np.zeros((128, nseg_max * 128), dtype=BF16)
        nv = node_of >= 0
        pp, bb = np.nonzero(nv)
        nodes_v = node_of[pp, bb]
        inv_tile[pp, bb] = invdeg[nodes_v]
        xdT[:, bb * 128 + pp] = x_bf[nodes_v].T

        ci["t1"] = t1
        ci["idx2"] = idx2
        ci["inv"] = inv_tile
        ci["xdT"] = xdT
        ci["node_of"] = node_of

    meta = {"nseg": nseg_max, "m1": m1_max, "m2": m2_max,
            "shard_rows": shard_rows, "cores": cores}
    return meta


def _build_program(meta):
    nseg, m1, m2 = meta["nseg"], meta["m1"], meta["m2"]
    shard_rows = meta["shard_rows"]
    (plan1, _m1), (plan2, _m2) = meta["cores"][0]["plans"]

    f32, bf16, i32 = mybir.dt.float32, mybir.dt.bfloat16, mybir.dt.int32
    nc = bacc.Bacc("TRN2", target_bir_lowering=False, debug=False,
                   num_devices=N_CORES)

    t1_d = nc.dram_tensor("t1", [128, m1, D], bf16, kind="ExternalInput")
    idx2_d = nc.dram_tensor("idx2", [128, m2], i32, kind="ExternalInput")
    inv_d = nc.dram_tensor("inv", [128, nseg], f32, kind="ExternalInput")
    xdT_d = nc.dram_tensor("xdT", [128, nseg * 128], bf16, kind="ExternalInput")
    w1l_d = nc.dram_tensor("w1lT", [128, 128], bf16, kind="ExternalInput")
    w1r_d = nc.dram_tensor("w1rT", [128, 128], bf16, kind="ExternalInput")
    w2l_d = nc.dram_tensor("w2lT", [128, 128], bf16, kind="ExternalInput")
    w2r_d = nc.dram_tensor("w2rT", [128, 128], bf16, kind="ExternalInput")
    b1_d = nc.dram_tensor("b1", [128, 1], f32, kind="ExternalInput")
    b2_d = nc.dram_tensor("b2", [128, 1], f32, kind="ExternalInput")
    outT_d = nc.dram_tensor("outT", [128, nseg * 128], f32, kind="ExternalOutput")

    h_shard = nc.dram_tensor("h_shard", [shard_rows, D], f32)
    h_full = nc.dram_tensor("h_full", [N_CORES * shard_rows, D], f32,
                            addr_space="Shared")

    with tile.TileContext(nc) as tc:
        with (
            tc.tile_pool(name="msg", bufs=2) as msg_pool,
            tc.tile_pool(name="persist", bufs=1) as pp,
            tc.tile_pool(name="work", bufs=3) as wp,
            tc.tile_pool(name="psum", bufs=2, space="PSUM") as psp,
        ):
            agg = pp.tile([128, nseg, D], f32, tag="agg")
            hT = pp.tile([128, nseg * 128], bf16, tag="hT")
            inv_t = pp.tile([128, nseg], f32, tag="inv")
            nc.sync.dma_start(out=inv_t[:], in_=inv_d.ap())
            idx2_t = pp.tile([128, m2], i32, tag="idx2")
            nc.sync.dma_start(out=idx2_t[:], in_=idx2_d.ap())
            w1l = pp.tile([128, 128], bf16, tag="w1l")
            nc.sync.dma_start(out=w1l[:], in_=w1l_d.ap())
            w1r = pp.tile([128, 128], bf16, tag="w1r")
            nc.sync.dma_start(out=w1r[:], in_=w1r_d.ap())
            w2l = pp.tile([128, 128], bf16, tag="w2l")
            nc.sync.dma_start(out=w2l[:], in_=w2l_d.ap())
            w2r = pp.tile([128, 128], bf16, tag="w2r")
            nc.sync.dma_start(out=w2r[:], in_=w2r_d.ap())
            b1_t = pp.tile([128, 1], f32, tag="b1")
            nc.sync.dma_start(out=b1_t[:], in_=b1_d.ap())
            b2_t = pp.tile([128, 1], f32, tag="b2")
            nc.sync.dma_start(out=b2_t[:], in_=b2_d.ap())
            ident = pp.tile([128, 128], f32, tag="ident")
            make_identity(nc, ident[:])

            def reduce_layer(plan, mc, get_chunk_tile):
                n_chunks = 0
                by_chunk = {}
                for (col, L, b) in plan:
                    k = col // mc
                    by_chunk.setdefault(k, []).append((col - k * mc, L, b))
                    n_chunks = max(n_chunks, k + 1)
                for k in range(n_chunks):
                    t = get_chunk_tile(k)
                    for (lc, L, b) in by_chunk.get(k, []):
                        view = t[:, lc:lc + L, :].rearrange(
                            "p (s l) f -> p s f l", s=1, l=L)
                        nc.vector.tensor_reduce(
                            out=agg[:, b:b + 1, :], in_=view,
                            axis=mybir.AxisListType.X, op=mybir.AluOpType.add)

            def block_phase(wl, wr, bias_t, rhs_r_fn, func, out_fn):
                for b in range(nseg):
                    mean_b = wp.tile([128, 128], f32, tag="mean")
                    nc.vector.tensor_scalar_mul(
                        mean_b[:], agg[:, b, :], inv_t[:, b:b + 1])
                    mT_ps = psp.tile([128, 128], f32, space="PSUM", tag="tp")
                    nc.tensor.transpose(out=mT_ps[:], in_=mean_b[:],
                                        identity=ident[:])
                    meanT = wp.tile([128, 128], bf16, tag="meanT")
                    nc.scalar.copy(meanT[:], mT_ps[:])
                    ps = psp.tile([128, 128], f32, space="PSUM", tag="mm")
                    nc.tensor.matmul(out=ps[:], lhsT=wl[:], rhs=meanT[:],
                                     start=True, stop=False)
                    nc.tensor.matmul(out=ps[:], lhsT=wr[:], rhs=rhs_r_fn(b),
                                     start=False, stop=True)
                    out_fn(b, ps, bias_t, func)

            # ---------------- layer 1 ----------------
            def l1_chunk(k):
                t = msg_pool.tile([128, MC1, D], bf16, tag="msg")
                nc.sync.dma_start(out=t[:], in_=t1_d.ap()[:, k * MC1:(k + 1) * MC1, :])
                return t

            reduce_layer(plan1, MC1, l1_chunk)

            def l1_rhs(b):
                xT_b = wp.tile([128, 128], bf16, tag="xTb")
                nc.sync.dma_start(out=xT_b[:], in_=xdT_d.ap()[:, b * 128:(b + 1) * 128])
                return xT_b[:]

            def l1_out(b, ps, bias_t, func):
                # hT (resident, bf16) and h rows (f32) for the shared table
                nc.scalar.activation(out=hT[:, b * 128:(b + 1) * 128], in_=ps[:],
                                     func=func, bias=bias_t[:], scale=1.0)
                hTf = wp.tile([128, 128], f32, tag="hTf")
                nc.scalar.activation(out=hTf[:], in_=ps[:], func=func,
                                     bias=bias_t[:], scale=1.0)
                hps = psp.tile([128, 128], f32, space="PSUM", tag="tp2")
                nc.tensor.transpose(out=hps[:], in_=hTf[:], identity=ident[:])
                h_blk = wp.tile([128, 128], f32, tag="hblk")
                nc.scalar.copy(h_blk[:], hps[:])
                nc.sync.dma_start(out=h_shard.ap()[b * 128:(b + 1) * 128, :],
                                  in_=h_blk[:])

            block_phase(w1l, w1r, b1_t, l1_rhs,
                        mybir.ActivationFunctionType.Relu, l1_out)

            zt = wp.tile([128, 128], f32, tag="zero")
            nc.vector.memset(zt[:], 0.0)
            nc.sync.dma_start(out=h_shard.ap()[nseg * 128:(nseg + 1) * 128, :],
                              in_=zt[:])

            nc.gpsimd.collective_compute(
                "AllGather", mybir.AluOpType.bypass,
                ins=[h_shard.ap().opt()], outs=[h_full.ap().opt()],
                replica_groups=[list(range(N_CORES))],
            )

            # ---------------- layer 2 ----------------
            def l2_chunk(k):
                t = msg_pool.tile([128, MC2, D], f32, tag="msg")
                for j in range(MC2):
                    nc.gpsimd.indirect_dma_start(
                        out=t[:, j, :], out_offset=None, in_=h_full.ap(),
                        in_offset=bass.IndirectOffsetOnAxis(
                            ap=idx2_t[:, k * MC2 + j:k * MC2 + j + 1], axis=0))
                return t

            reduce_layer(plan2, MC2, l2_chunk)

            def l2_rhs(b):
                return hT[:, b * 128:(b + 1) * 128]

            def l2_out(b, ps, bias_t, func):
                oT = wp.tile([128, 128], f32, tag="oT")
                nc.scalar.activation(out=oT[:], in_=ps[:], func=func,
                                     bias=bias_t[:], scale=1.0)
                nc.sync.dma_start(out=outT_d.ap()[:, b * 128:(b + 1) * 128],
                                  in_=oT[:])

            block_phase(w2l, w2r, b2_t, l2_rhs,
                        mybir.ActivationFunctionType.Identity, l2_out)

    nc.compile()
    return nc


_CACHE = {}


def kernel(x, edge_index, W1_l, b1_l, W1_r, W2_l, b2_l, W2_r):
    x = np.asarray(x, dtype=np.float32)
    meta = _build_tables(x, np.asarray(edge_index))

    key = (meta["nseg"], meta["m1"], meta["m2"])
    if key not in _CACHE:
        _CACHE[key] = _build_program(meta)
    nc = _CACHE[key]

    in_maps = []
    for c in range(N_CORES):
        ci = meta["cores"][c]
        in_maps.append({
            "t1": ci["t1"], "idx2": ci["idx2"], "inv": ci["inv"],
            "xdT": ci["xdT"],
            "w1lT": np.asarray(W1_l, np.float32).T.astype(BF16).copy(),
            "w1rT": np.asarray(W1_r, np.float32).T.astype(BF16).copy(),
            "w2lT": np.asarray(W2_l, np.float32).T.astype(BF16).copy(),
            "w2rT": np.asarray(W2_r, np.float32).T.astype(BF16).copy(),
            "b1": np.asarray(b1_l, np.float32).reshape(128, 1).copy(),
            "b2": np.asarray(b2_l, np.float32).reshape(128, 1).copy(),
        })

    res = run_bass_kernel_spmd(nc, in_maps, core_ids=list(range(N_CORES)))

    out = np.zeros((N_NODES, D), dtype=np.float32)
    for c in range(N_CORES):
        outT = res.results[c]["outT"].reshape(128, meta["nseg"] * 128)
        node_of = meta["cores"][c]["node_of"]      # [128, nseg]
        pp_, bb = np.nonzero(node_of >= 0)
        nodes = node_of[pp_, bb]
        out[nodes] = outT[:, bb * 128 + pp_].T
    return out

